# revision 17
# baseline (speedup 1.0000x reference)
"""Trainium2 Bass kernel for nn_ContrastLoss (contrastive PSD loss).

Math notes (validated against the jax reference and a numpy emulator):
  * The band (rfft bins 92..568 of a 4096-point DFT) excludes DC, so the
    mean subtraction in the reference is a no-op for the band PSD.
  * diag(D) == 0 for the pairwise-MSE matrix, and every _compare() term
    reduces to rank-1 statistics of the normalized PSD matrices:
        sum_ij D_ij * F = M*SSQ_a + N*SSQ_b - 2 * cs_a . cs_b
    with SSQ = sum of squared entries and cs = column sums.  So the NxN
    Gram matrix is never materialized; the device only produces per-core
    column sums and per-row (sum, sum-of-squares) statistics.
  * Radix-2 DIF recursion on the crop halves splits the band bins into
    classes by k mod 32 with REAL sub-signals (x0+-x1 folds, applied
    where the class phases allow it):
        od    k odd       : d     [2048]   238 bins
        eo    k = 2 mod 4 : eo    [1024]   119 bins
        eeo   k = 4 mod 8 : eeo   [512]     60 bins
        eeeo  k = 8 mod 16: eeeo  [256]     30 bins
        eeeee k = 0 mod 32: eeeee [128]     15 bins
        eeeeo k =16 mod 32: eeeeo [128]     15 bins
  * Universal symmetric fold: for each class, cos(theta(n-m,k)) =
    -cos(theta(m,k)) and sin(theta(n-m,k)) = +sin(theta(m,k)), so
        Re X = [s_0, s_m - s_{n-m}] . cos-matrix     (n/2 contraction)
        Im X = [s_{n/2}, s_m + s_{n-m}] . -sin-matrix (n/2 contraction)
    (edge samples ride in the free m=0 slot; cos theta(n/2,k) = 0 and
    sin theta(0,k) = 0 for these classes).  This halves the matmul MACs
    again: 2620 PE cycles per 128-crop block, DFT matrices 655 KB.
  * All matmul operands are fp8-e4m3; >=256-contractions use DoubleRow
    perf mode (two 128-deep k-tiles per instruction).  End-to-end e4m3
    error: ~4e-6 on the loss terms, ~3e-3 on the cancellation-dominated
    total loss, far under the 2e-2 gate.

Device schedule per core (1024 crops of the 8192 total):
  Big-line DMAs (one descriptor per partition): W blob [128,2,2560]
  (5 KB lines) then 8 crop blocks [128,32,128] (4 KB lines).  Dummy fp8
  matmuls pre-ramp the PE p-state while DMA streams.  Per block: 20
  matmuls -> 2 PSUM tiles, ACT Square -> sq, DVE adds -> band PSD
  p [128,477], DVE row-reduce -> rowsum, ACT Square+accum -> rowsumsq,
  PE colsum matmul with lhsT = 1/rowsum -> cs [1,477].  Host combines
  the 8 cores' (cs, rowstats) in float64.
"""

import numpy as np

# Problem constants (hardcoded; kernel.py must be self-contained)
B, C, T = 2, 64, 32768
L = 4096
K_CROPS = 32
HALF = L // 2                  # 2048
N_ROWS = C * K_CROPS           # 2048 rows per PSD matrix
N_CORES = 8
ROWS_PER_CORE = N_ROWS * 4 // N_CORES   # 1024
NB = ROWS_PER_CORE // 128      # 8 row blocks per core
NSLOT = 32                     # 128-sample slots per crop
N_WARM = 22                    # PE p-state pre-ramp matmuls

# Band-bin classes: (kset, signal length n, folded?)
K_EEEEE = np.arange(96, 545, 32)    # 15 bins, k=0 mod 32
K_EEEEO = np.arange(112, 561, 32)   # 15 bins, k=16 mod 32
K_EEEO = np.arange(104, 569, 16)    # 30 bins, k=8 mod 16
K_EEO = np.arange(92, 565, 8)       # 60 bins, k=4 mod 8
K_EO = np.arange(94, 567, 4)        # 119 bins, k=2 mod 4
K_OD = np.arange(93, 568, 2)        # 238 bins, k odd
F = 477
FP = F + 1                     # pad col so fp32r matmul free dim is even
WX = 2560                      # W blob free columns (per k-tile slot)

_NC = None
_W_CACHE = None


def _w_blob():
    """fp8-e4m3 DFT matrices packed as [128, 2, WX].

    dim1 indexes the two k-tiles of a DoubleRow pair (or t=0/1 reuse for
    the four plain 128-contraction classes).  Layout in the X dim:
      [0:30)    t0 eeeee [cos|-sin], t1 eeeeo
      [30:60)   t0 eeeo-cos,         t1 eeeo-sin
      [60:120)  eeo-cos   [120:180) eeo-sin      (1 pair)
      [180:418) eo-cos x2 [418:656) eo-sin x2    (119 cols per pair)
      [656:1608) od-cos x4 [1608:2560) od-sin x4 (238 cols per pair)
    """
    global _W_CACHE
    if _W_CACHE is not None:
        return _W_CACHE
    import ml_dtypes

    def ang(ms, ks):
        return 2.0 * np.pi * (ms[:, None].astype(np.float64) *
                              ks[None, :]) / float(L)

    blob = np.zeros((128, 2, WX), dtype=np.float32)

    def put_plain(t, x0, ks):
        a = ang(np.arange(128), ks)
        blob[:, t, x0:x0 + 2 * len(ks)] = np.concatenate(
            [np.cos(a), -np.sin(a)], axis=1)

    def put_fold(x0c, x0s, ks, n):
        h = n // 2
        mc = np.concatenate([[0], np.arange(1, h)])
        ms = np.concatenate([[h], np.arange(1, h)])
        wc = np.cos(ang(mc, ks))           # [h, nb]
        ws = -np.sin(ang(ms, ks))
        npair = h // 256
        nb = len(ks)
        for w, x0 in ((wc, x0c), (ws, x0s)):
            r = w.reshape(npair, 2, 128, nb).transpose(2, 1, 0, 3)
            blob[:, :, x0:x0 + npair * nb] = r.reshape(128, 2, npair * nb)

    put_plain(0, 0, K_EEEEE)
    put_plain(1, 0, K_EEEEO)
    # eeeo: folded to 128-contraction -> plain slots t0/t1 at [30:60)
    h = 128
    mc = np.concatenate([[0], np.arange(1, h)])
    ms = np.concatenate([[h], np.arange(1, h)])
    blob[:, 0, 30:60] = np.cos(ang(mc, K_EEEO))
    blob[:, 1, 30:60] = -np.sin(ang(ms, K_EEEO))
    put_fold(60, 120, K_EEO, 512)
    put_fold(180, 418, K_EO, 1024)
    put_fold(656, 1608, K_OD, 2048)

    _W_CACHE = np.ascontiguousarray(blob.astype(ml_dtypes.float8_e4m3))
    return _W_CACHE


def _build_module():
    global _NC
    if _NC is not None:
        return _NC
    import concourse.bacc as bacc
    import concourse.bass as bass
    import concourse.tile as tile
    from concourse import mybir

    f32 = mybir.dt.float32
    f32r = mybir.dt.float32r
    f8 = mybir.dt.float8e4
    AF = mybir.ActivationFunctionType
    DR = mybir.MatmulPerfMode.DoubleRow

    nc = bacc.Bacc("TRN2", target_bir_lowering=False, debug=False,
                   num_devices=N_CORES)

    # crops: [blk, partition, slot, crop] fp8, 4 KB lines
    crops_d = nc.dram_tensor("crops", [NB, 128, NSLOT, 128], f8,
                             kind="ExternalInput")
    w_d = nc.dram_tensor("w", [128, 2, WX], f8, kind="ExternalInput")
    out_cs = nc.dram_tensor("out_cs", [1, F], f32, kind="ExternalOutput")
    out_rq = nc.dram_tensor("out_rq", [128, 6 * NB], f32,
                            kind="ExternalOutput")

    with tile.TileContext(nc) as tc:
        with (
            tc.tile_pool(name="wp", bufs=1) as wp,
            tc.tile_pool(name="cp", bufs=NB) as cp,
            tc.tile_pool(name="sq", bufs=4) as sqp,
            tc.tile_pool(name="pp", bufs=3) as ppool,
            tc.tile_pool(name="sm", bufs=8) as sm,
            tc.tile_pool(name="outp", bufs=1) as outp,
            tc.tile_pool(name="ps", bufs=6, space=bass.MemorySpace.PSUM) as ps,
            tc.tile_pool(name="psw", bufs=1, space=bass.MemorySpace.PSUM) as psw,
            tc.tile_pool(name="pcs", bufs=1, space=bass.MemorySpace.PSUM) as pcs,
        ):
            wb = wp.tile([128, 2, WX], f8)
            rq_t = outp.tile([128, 6 * NB], f32)
            zero_col = outp.tile([128, 1], f32)
            warm_t = outp.tile([128, 2, 256], f8)
            nc.vector.memset(zero_col, 0.0)
            nc.vector.memset(warm_t, 0.0)
            cs_psum = pcs.tile([1, FP], f32)
            warm_ps = psw.tile([128, 256], f32)

            # PE p-state pre-ramp while DMA streams.
            for i in range(N_WARM):
                nc.tensor.matmul(warm_ps, warm_t[:, :, 0:128], warm_t,
                                 start=True, stop=True, perf_mode=DR)

            # DMAs in consumption order (big descriptors, one hw queue)
            nc.sync.dma_start(out=wb, in_=w_d[:])
            cpb = []
            for blk in range(NB):
                ct = cp.tile([128, NSLOT, 128], f8, tag="cp", name=f"c{blk}")
                cpb.append(ct)
                nc.sync.dma_start(out=ct, in_=crops_d[blk])

            for blk in range(NB):
                cb = cpb[blk]
                ev_t = ps.tile([128, 478], f32, tag="ps", name=f"ev{blk}")
                od_t = ps.tile([128, 476], f32, tag="ps", name=f"od{blk}")
                # od first so the tail of the last block is the short
                # ev post-chain, not the long od one.
                for px0, s0, wx0 in ((0, 16, 656), (238, 24, 1608)):
                    for c in range(4):
                        nc.tensor.matmul(
                            od_t[:, px0:px0 + 238],
                            cb[:, s0 + 2 * c:s0 + 2 * c + 2, :],
                            wb[:, :, wx0 + 238 * c:wx0 + 238 * (c + 1)],
                            start=(c == 0), stop=(c == 3), perf_mode=DR)
                # plain 128-contraction classes (single k-tile)
                for x0, slot, t, wx0 in ((0, 0, 0, 0), (30, 1, 1, 0),
                                         (60, 2, 0, 30), (90, 3, 1, 30)):
                    nc.tensor.matmul(ev_t[:, x0:x0 + 30], cb[:, slot, :],
                                     wb[:, t, wx0:wx0 + 30],
                                     start=True, stop=True)
                # DoubleRow folded ev classes
                for px0, s0, wx0, nb, npair in (
                        (120, 4, 60, 60, 1), (180, 6, 120, 60, 1),
                        (240, 8, 180, 119, 2), (359, 12, 418, 119, 2)):
                    for c in range(npair):
                        nc.tensor.matmul(
                            ev_t[:, px0:px0 + nb],
                            cb[:, s0 + 2 * c:s0 + 2 * c + 2, :],
                            wb[:, :, wx0 + nb * c:wx0 + nb * (c + 1)],
                            start=(c == 0), stop=(c == npair - 1),
                            perf_mode=DR)

                # Post: PSD, row stats (bn_stats), normalized column sums.
                sq_ev = sqp.tile([128, 478], f32, tag="sqe", name=f"se{blk}")
                sq_od = sqp.tile([128, 476], f32, tag="sqd", name=f"sd{blk}")
                nc.scalar.activation(out=sq_od, in_=od_t, func=AF.Square)
                nc.scalar.activation(out=sq_ev, in_=ev_t, func=AF.Square)
                p_t = ppool.tile([128, FP], f32r, tag="p", name=f"p{blk}")
                with nc.allow_low_precision(reason="fp32r is fp32-width"):
                    # od add on the (otherwise idle) gpsimd engine
                    nc.gpsimd.tensor_add(p_t[:, 239:477], sq_od[:, 0:238],
                                         sq_od[:, 238:476])
                    for dst, a, b_ in ((0, 0, 15), (15, 30, 45),
                                       (30, 60, 90), (60, 120, 180),
                                       (120, 240, 359)):
                        n = b_ - a
                        nc.vector.tensor_add(p_t[:, dst:dst + n],
                                             sq_ev[:, a:a + n],
                                             sq_ev[:, b_:b_ + n])
                    nc.vector.tensor_copy(p_t[:, F:FP], zero_col)
                # bn_stats: [cnt_e, mean_e, cnt*var_e, cnt_o, mean_o,
                # cnt*var_o] over even/odd-indexed band bins; the host
                # reconstructs rowsum and sum(p^2) exactly.
                bn = rq_t[:, 6 * blk:6 * blk + 6]
                nc.vector.bn_stats(bn, p_t[:, 0:F])
                # rs/238 = mean_e*(239/238) + mean_o; colsum lhsT is then
                # 238/rs and the host divides cs by 238.
                rsn = sm.tile([128, 1], f32, tag="rsn", name=f"r{blk}")
                nc.vector.scalar_tensor_tensor(
                    out=rsn, in0=bn[:, 1:2], scalar=239.0 / 238.0,
                    in1=bn[:, 4:5], op0=mybir.AluOpType.mult,
                    op1=mybir.AluOpType.add)
                inv = sm.tile([128, 1], f32r, tag="inv", name=f"i{blk}")
                with nc.allow_low_precision(reason="fp32r is fp32-width"):
                    nc.vector.reciprocal(inv, rsn)
                nc.tensor.matmul(cs_psum, inv, p_t,
                                 start=(blk == 0), stop=(blk == NB - 1))

            cs_sb = outp.tile([1, F], f32)
            nc.vector.tensor_copy(cs_sb, cs_psum[:, 0:F])
            nc.sync.dma_start(out=out_cs[:], in_=cs_sb)
            nc.sync.dma_start(out=out_rq[:], in_=rq_t)

    nc.compile()
    _NC = nc
    return nc


def _fold_cs(sig):
    """sig [R, n] -> (cos fold [R, n/2], sin fold [R, n/2])."""
    n = sig.shape[1]
    h = n // 2
    c = np.empty((sig.shape[0], h), dtype=np.float32)
    s = np.empty_like(c)
    c[:, 0] = sig[:, 0]
    s[:, 0] = sig[:, h]
    c[:, 1:] = sig[:, 1:h] - sig[:, :h:-1]
    s[:, 1:] = sig[:, 1:h] + sig[:, :h:-1]
    return c, s


def _core_input(rows_ed):
    """rows_ed: (e, d) each [1024, 2048] f32 -> crops [8,128,32,128] fp8."""
    import ml_dtypes
    e, d = rows_ed
    ee = e[:, :1024] + e[:, 1024:]
    eo = e[:, :1024] - e[:, 1024:]
    eee = ee[:, :512] + ee[:, 512:]
    eeo = ee[:, :512] - ee[:, 512:]
    eeee = eee[:, :256] + eee[:, 256:]
    eeeo = eee[:, :256] - eee[:, 256:]
    eeeee = eeee[:, :128] + eeee[:, 128:]
    eeeeo = eeee[:, :128] - eeee[:, 128:]
    parts = [eeeee, eeeeo]
    for sig in (eeeo, eeo, eo, d):
        parts.extend(_fold_cs(sig))
    q = np.concatenate(parts, axis=1).astype(ml_dtypes.float8_e4m3)
    # [128b+cr, 128s+p] -> [b, p, s, cr]
    arr = q.reshape(NB, 128, NSLOT, 128).transpose(0, 3, 2, 1)
    return {"crops": np.ascontiguousarray(arr)}


def _host_prepare(model_output, GT_sig, offsets_st, offsets_t):
    """Build per-core in_maps."""
    w_blob = _w_blob()
    from numpy.lib.stride_tricks import sliding_window_view
    in_maps = []
    mats = []   # 4 matrices' (e, d) row data [2048, 2048] each
    for b in range(B):
        offs = np.asarray(offsets_st[b], dtype=np.int64).reshape(-1)
        ch_idx = np.repeat(np.arange(C), K_CROPS)
        base = np.asarray(model_output[b], dtype=np.float32)
        win = sliding_window_view(base, L, axis=-1)  # [C, T-L+1, L]
        cr = win[ch_idx, offs]                       # [2048, L]
        mats.append((cr[:, :HALF] + cr[:, HALF:],
                     cr[:, :HALF] - cr[:, HALF:]))
    for b in range(B):
        offs = np.asarray(offsets_t[b], dtype=np.int64).reshape(-1)
        win = sliding_window_view(
            np.asarray(GT_sig[b], dtype=np.float32), L)
        cr = win[offs]
        mats.append((cr[:, :HALF] + cr[:, HALF:],
                     cr[:, :HALF] - cr[:, HALF:]))
    for m in range(4):
        e, d = mats[m]
        for h in range(2):
            sl = slice(h * ROWS_PER_CORE, (h + 1) * ROWS_PER_CORE)
            im = {"w": w_blob}
            im.update(_core_input((e[sl], d[sl])))
            in_maps.append(im)
    return in_maps


def _combine(results, label_flag):
    """results: list of 8 dicts with out_cs [1,F], out_rq [128,6*NB].

    out_rq holds bn_stats fields per block: [cnt_e, mean_e, cnt*var_e,
    cnt_o, mean_o, cnt*var_o] over the even/odd-indexed band bins of p.
    rowsum = cnt_e*mean_e + cnt_o*mean_o; sum(p^2) = (cnt*var + cnt*
    mean^2) summed over both halves.  Device cs used lhsT = 238/rowsum.
    """
    cs = np.zeros((4, F), dtype=np.float64)
    ssq = np.zeros(4, dtype=np.float64)
    for m in range(4):
        for h in range(2):
            r = results[2 * m + h]
            cs[m] += np.asarray(r["out_cs"], dtype=np.float64)[0] / 238.0
            rq = np.asarray(r["out_rq"], dtype=np.float64)
            ne, me, ve = rq[:, 0::6], rq[:, 1::6], rq[:, 2::6]
            no, mo, vo = rq[:, 3::6], rq[:, 4::6], rq[:, 5::6]
            rs = ne * me + no * mo
            q = (ve + ne * me * me) + (vo + no * mo * mo)
            ssq[m] += float(np.sum(q / (rs * rs)))

    N = float(N_ROWS)

    def cmp_excl(a):
        return (2.0 * N * ssq[a] - 2.0 * np.dot(cs[a], cs[a])) / F / (N * (N - 1.0))

    def cmp_full(a, b):
        return (N * ssq[a] + N * ssq[b] - 2.0 * np.dot(cs[a], cs[b])) / F / (N * N)

    lf = np.asarray(label_flag, dtype=np.float64).reshape(-1)
    lf_sum = lf[0] + lf[1]
    denom = 1.0 if lf_sum == 0 else lf_sum
    pos_loss = (cmp_excl(0) + cmp_excl(1)) / 2.0
    neg_loss = -cmp_full(0, 1)
    pos_GT = (lf[0] * cmp_full(0, 2) + lf[1] * cmp_full(1, 3)) / denom
    neg_GT = -(lf[0] * cmp_full(1, 2) + lf[1] * cmp_full(0, 3)) / denom
    if lf_sum == 0:
        pos_GT = 0.0
        neg_GT = 0.0
    loss = pos_loss + neg_loss + pos_GT + neg_GT
    return (np.float32(loss), np.float32(pos_loss), np.float32(neg_loss),
            np.float32(pos_GT), np.float32(neg_GT))


def run(inputs, trace=False):
    """Returns (outputs_tuple, BassKernelResults)."""
    from concourse import bass_utils
    nc = _build_module()
    in_maps = _host_prepare(
        inputs["model_output"], inputs["GT_sig"],
        inputs["offsets_st"], inputs["offsets_t"])
    res = bass_utils.run_bass_kernel_spmd(
        nc, in_maps, core_ids=list(range(N_CORES)), trace=trace)
    outs = _combine(res.results, inputs["label_flag"])
    return outs, res


def kernel(**inputs):
    outs, _ = run(inputs)
    return outs


# revision 18
# speedup vs baseline: 1.0461x; 1.0461x over previous
"""Trainium2 Bass kernel for nn_ContrastLoss (contrastive PSD loss).

Math notes (validated against the jax reference and a numpy emulator):
  * The band (rfft bins 92..568 of a 4096-point DFT) excludes DC, so the
    mean subtraction in the reference is a no-op for the band PSD.
  * diag(D) == 0 for the pairwise-MSE matrix, and every _compare() term
    reduces to rank-1 statistics of the normalized PSD matrices:
        sum_ij D_ij * F = M*SSQ_a + N*SSQ_b - 2 * cs_a . cs_b
    with SSQ = sum of squared entries and cs = column sums.  So the NxN
    Gram matrix is never materialized; the device only produces per-core
    column sums and per-row (sum, sum-of-squares) statistics.
  * Radix-2 DIF recursion on the crop halves splits the band bins into
    classes by k mod 32 with REAL sub-signals (x0+-x1 folds, applied
    where the class phases allow it):
        od    k odd       : d     [2048]   238 bins
        eo    k = 2 mod 4 : eo    [1024]   119 bins
        eeo   k = 4 mod 8 : eeo   [512]     60 bins
        eeeo  k = 8 mod 16: eeeo  [256]     30 bins
        eeeee k = 0 mod 32: eeeee [128]     15 bins
        eeeeo k =16 mod 32: eeeeo [128]     15 bins
  * Universal symmetric fold: for each class, cos(theta(n-m,k)) =
    -cos(theta(m,k)) and sin(theta(n-m,k)) = +sin(theta(m,k)), so
        Re X = [s_0, s_m - s_{n-m}] . cos-matrix     (n/2 contraction)
        Im X = [s_{n/2}, s_m + s_{n-m}] . -sin-matrix (n/2 contraction)
    (edge samples ride in the free m=0 slot; cos theta(n/2,k) = 0 and
    sin theta(0,k) = 0 for these classes).  This halves the matmul MACs
    again: 2620 PE cycles per 128-crop block, DFT matrices 655 KB.
  * All matmul operands are fp8-e4m3; >=256-contractions use DoubleRow
    perf mode (two 128-deep k-tiles per instruction).  End-to-end e4m3
    error: ~4e-6 on the loss terms, ~3e-3 on the cancellation-dominated
    total loss, far under the 2e-2 gate.

Device schedule per core (1024 crops of the 8192 total):
  Big-line DMAs (one descriptor per partition): W blob [128,2,2560]
  (5 KB lines) then 8 crop blocks [128,32,128] (4 KB lines).  Dummy fp8
  matmuls pre-ramp the PE p-state while DMA streams.  Per block: 20
  matmuls -> 2 PSUM tiles, ACT Square -> sq, DVE adds -> band PSD
  p [128,477], DVE row-reduce -> rowsum, ACT Square+accum -> rowsumsq,
  PE colsum matmul with lhsT = 1/rowsum -> cs [1,477].  Host combines
  the 8 cores' (cs, rowstats) in float64.
"""

import numpy as np

# Problem constants (hardcoded; kernel.py must be self-contained)
B, C, T = 2, 64, 32768
L = 4096
K_CROPS = 32
HALF = L // 2                  # 2048
N_ROWS = C * K_CROPS           # 2048 rows per PSD matrix
N_CORES = 8
ROWS_PER_CORE = N_ROWS * 4 // N_CORES   # 1024
NB = ROWS_PER_CORE // 128      # 8 row blocks per core
NSLOT = 32                     # 128-sample slots per crop
N_WARM = 22                    # PE p-state pre-ramp matmuls

# Band-bin classes: (kset, signal length n, folded?)
K_EEEEE = np.arange(96, 545, 32)    # 15 bins, k=0 mod 32
K_EEEEO = np.arange(112, 561, 32)   # 15 bins, k=16 mod 32
K_EEEO = np.arange(104, 569, 16)    # 30 bins, k=8 mod 16
K_EEO = np.arange(92, 565, 8)       # 60 bins, k=4 mod 8
K_EO = np.arange(94, 567, 4)        # 119 bins, k=2 mod 4
K_OD = np.arange(93, 568, 2)        # 238 bins, k odd
F = 477
FP = F + 1                     # pad col so fp32r matmul free dim is even
WX = 2560                      # W blob free columns (per k-tile slot)

_NC = None
_W_CACHE = None


def _w_blob():
    """fp8-e4m3 DFT matrices packed as [128, 2, WX].

    dim1 indexes the two k-tiles of a DoubleRow pair (or t=0/1 reuse for
    the four plain 128-contraction classes).  Layout in the X dim:
      [0:30)    t0 eeeee [cos|-sin], t1 eeeeo
      [30:60)   t0 eeeo-cos,         t1 eeeo-sin
      [60:120)  eeo-cos   [120:180) eeo-sin      (1 pair)
      [180:418) eo-cos x2 [418:656) eo-sin x2    (119 cols per pair)
      [656:1608) od-cos x4 [1608:2560) od-sin x4 (238 cols per pair)
    """
    global _W_CACHE
    if _W_CACHE is not None:
        return _W_CACHE
    import ml_dtypes

    def ang(ms, ks):
        return 2.0 * np.pi * (ms[:, None].astype(np.float64) *
                              ks[None, :]) / float(L)

    blob = np.zeros((128, 2, WX), dtype=np.float32)

    def put_plain(t, x0, ks):
        a = ang(np.arange(128), ks)
        blob[:, t, x0:x0 + 2 * len(ks)] = np.concatenate(
            [np.cos(a), -np.sin(a)], axis=1)

    def put_fold(x0c, x0s, ks, n):
        h = n // 2
        mc = np.concatenate([[0], np.arange(1, h)])
        ms = np.concatenate([[h], np.arange(1, h)])
        wc = np.cos(ang(mc, ks))           # [h, nb]
        ws = -np.sin(ang(ms, ks))
        npair = h // 256
        nb = len(ks)
        for w, x0 in ((wc, x0c), (ws, x0s)):
            r = w.reshape(npair, 2, 128, nb).transpose(2, 1, 0, 3)
            blob[:, :, x0:x0 + npair * nb] = r.reshape(128, 2, npair * nb)

    put_plain(0, 0, K_EEEEE)
    put_plain(1, 0, K_EEEEO)
    # eeeo: folded to 128-contraction -> plain slots t0/t1 at [30:60)
    h = 128
    mc = np.concatenate([[0], np.arange(1, h)])
    ms = np.concatenate([[h], np.arange(1, h)])
    blob[:, 0, 30:60] = np.cos(ang(mc, K_EEEO))
    blob[:, 1, 30:60] = -np.sin(ang(ms, K_EEEO))
    put_fold(60, 120, K_EEO, 512)
    put_fold(180, 418, K_EO, 1024)
    put_fold(656, 1608, K_OD, 2048)

    _W_CACHE = np.ascontiguousarray(blob.astype(ml_dtypes.float8_e4m3))
    return _W_CACHE


def _build_module():
    global _NC
    if _NC is not None:
        return _NC
    import concourse.bacc as bacc
    import concourse.bass as bass
    import concourse.tile as tile
    from concourse import mybir

    f32 = mybir.dt.float32
    f32r = mybir.dt.float32r
    f8 = mybir.dt.float8e4
    AF = mybir.ActivationFunctionType
    DR = mybir.MatmulPerfMode.DoubleRow

    nc = bacc.Bacc("TRN2", target_bir_lowering=False, debug=False,
                   num_devices=N_CORES)

    # crops: [blk, partition, slot, crop] fp8, 4 KB lines
    crops_d = nc.dram_tensor("crops", [NB, 128, NSLOT, 128], f8,
                             kind="ExternalInput")
    w_d = nc.dram_tensor("w", [128, 2, WX], f8, kind="ExternalInput")
    out_cs = nc.dram_tensor("out_cs", [1, F], f32, kind="ExternalOutput")
    out_rq = nc.dram_tensor("out_rq", [128, 6 * NB], f32,
                            kind="ExternalOutput")

    with tile.TileContext(nc) as tc:
        with (
            tc.tile_pool(name="wp", bufs=1) as wp,
            tc.tile_pool(name="cp", bufs=NB) as cp,
            tc.tile_pool(name="sq", bufs=4) as sqp,
            tc.tile_pool(name="pp", bufs=3) as ppool,
            tc.tile_pool(name="sm", bufs=8) as sm,
            tc.tile_pool(name="outp", bufs=1) as outp,
            tc.tile_pool(name="ps", bufs=6, space=bass.MemorySpace.PSUM) as ps,
            tc.tile_pool(name="psw", bufs=1, space=bass.MemorySpace.PSUM) as psw,
            tc.tile_pool(name="pcs", bufs=1, space=bass.MemorySpace.PSUM) as pcs,
        ):
            wb = wp.tile([128, 2, WX], f8)
            rq_t = outp.tile([128, 6 * NB], f32)
            zero_col = outp.tile([128, 1], f32)
            warm_t = outp.tile([128, 2, 256], f8)
            nc.vector.memset(zero_col, 0.0)
            nc.vector.memset(warm_t, 0.0)
            cs_psum = pcs.tile([1, FP], f32)
            warm_ps = psw.tile([128, 256], f32)

            # PE p-state pre-ramp while DMA streams.
            for i in range(N_WARM):
                nc.tensor.matmul(warm_ps, warm_t[:, :, 0:128], warm_t,
                                 start=True, stop=True, perf_mode=DR)

            # DMAs in consumption order (big descriptors, one hw queue)
            nc.sync.dma_start(out=wb, in_=w_d[:])
            cpb = []
            for blk in range(NB):
                ct = cp.tile([128, NSLOT, 128], f8, tag="cp", name=f"c{blk}")
                cpb.append(ct)
                nc.sync.dma_start(out=ct, in_=crops_d[blk])

            for blk in range(NB):
                cb = cpb[blk]
                ev_t = ps.tile([128, 478], f32, tag="ps", name=f"ev{blk}")
                od_t = ps.tile([128, 476], f32, tag="ps", name=f"od{blk}")
                # od first so the tail of the last block is the short
                # ev post-chain, not the long od one.
                for px0, s0, wx0 in ((0, 16, 656), (238, 24, 1608)):
                    for c in range(4):
                        nc.tensor.matmul(
                            od_t[:, px0:px0 + 238],
                            cb[:, s0 + 2 * c:s0 + 2 * c + 2, :],
                            wb[:, :, wx0 + 238 * c:wx0 + 238 * (c + 1)],
                            start=(c == 0), stop=(c == 3), perf_mode=DR)
                # ev_t layout: [cos-half 0:239 | sin-half 239:478] so the
                # PSD assembly is a single DVE add per tile.
                # plain 128-contraction classes (single k-tile)
                for x0, slot, t, wx0 in ((0, 0, 0, 0), (239, 0, 0, 15),
                                         (15, 1, 1, 0), (254, 1, 1, 15)):
                    nc.tensor.matmul(ev_t[:, x0:x0 + 15], cb[:, slot, :],
                                     wb[:, t, wx0:wx0 + 15],
                                     start=True, stop=True)
                for x0, slot, t in ((30, 2, 0), (269, 3, 1)):
                    nc.tensor.matmul(ev_t[:, x0:x0 + 30], cb[:, slot, :],
                                     wb[:, t, 30:60],
                                     start=True, stop=True)
                # DoubleRow folded ev classes
                for px0, s0, wx0, nb, npair in (
                        (60, 4, 60, 60, 1), (299, 6, 120, 60, 1),
                        (120, 8, 180, 119, 2), (359, 12, 418, 119, 2)):
                    for c in range(npair):
                        nc.tensor.matmul(
                            ev_t[:, px0:px0 + nb],
                            cb[:, s0 + 2 * c:s0 + 2 * c + 2, :],
                            wb[:, :, wx0 + nb * c:wx0 + nb * (c + 1)],
                            start=(c == 0), stop=(c == npair - 1),
                            perf_mode=DR)

                # Post: PSD, row stats (bn_stats), normalized column sums.
                sq_ev = sqp.tile([128, 478], f32, tag="sqe", name=f"se{blk}")
                sq_od = sqp.tile([128, 476], f32, tag="sqd", name=f"sd{blk}")
                nc.scalar.activation(out=sq_od, in_=od_t, func=AF.Square)
                nc.scalar.activation(out=sq_ev, in_=ev_t, func=AF.Square)
                p_t = ppool.tile([128, FP], f32r, tag="p", name=f"p{blk}")
                with nc.allow_low_precision(reason="fp32r is fp32-width"):
                    nc.vector.tensor_add(p_t[:, 239:477], sq_od[:, 0:238],
                                         sq_od[:, 238:476])
                    nc.vector.tensor_add(p_t[:, 0:239], sq_ev[:, 0:239],
                                         sq_ev[:, 239:478])
                    nc.vector.tensor_copy(p_t[:, F:FP], zero_col)
                # bn_stats: [cnt_e, mean_e, cnt*var_e, cnt_o, mean_o,
                # cnt*var_o] over even/odd-indexed band bins; the host
                # reconstructs rowsum and sum(p^2) exactly.
                bn = rq_t[:, 6 * blk:6 * blk + 6]
                nc.vector.bn_stats(bn, p_t[:, 0:F])
                # rs/238 = mean_e*(239/238) + mean_o; colsum lhsT is then
                # 238/rs and the host divides cs by 238.
                rsn = sm.tile([128, 1], f32, tag="rsn", name=f"r{blk}")
                nc.vector.scalar_tensor_tensor(
                    out=rsn, in0=bn[:, 1:2], scalar=239.0 / 238.0,
                    in1=bn[:, 4:5], op0=mybir.AluOpType.mult,
                    op1=mybir.AluOpType.add)
                inv = sm.tile([128, 1], f32r, tag="inv", name=f"i{blk}")
                with nc.allow_low_precision(reason="fp32r is fp32-width"):
                    nc.vector.reciprocal(inv, rsn)
                nc.tensor.matmul(cs_psum, inv, p_t,
                                 start=(blk == 0), stop=(blk == NB - 1))

            cs_sb = outp.tile([1, F], f32)
            nc.vector.tensor_copy(cs_sb, cs_psum[:, 0:F])
            nc.sync.dma_start(out=out_cs[:], in_=cs_sb)
            nc.sync.dma_start(out=out_rq[:], in_=rq_t)

    nc.compile()
    _NC = nc
    return nc


def _fold_cs(sig):
    """sig [R, n] -> (cos fold [R, n/2], sin fold [R, n/2])."""
    n = sig.shape[1]
    h = n // 2
    c = np.empty((sig.shape[0], h), dtype=np.float32)
    s = np.empty_like(c)
    c[:, 0] = sig[:, 0]
    s[:, 0] = sig[:, h]
    c[:, 1:] = sig[:, 1:h] - sig[:, :h:-1]
    s[:, 1:] = sig[:, 1:h] + sig[:, :h:-1]
    return c, s


def _core_input(rows_ed):
    """rows_ed: (e, d) each [1024, 2048] f32 -> crops [8,128,32,128] fp8."""
    import ml_dtypes
    e, d = rows_ed
    ee = e[:, :1024] + e[:, 1024:]
    eo = e[:, :1024] - e[:, 1024:]
    eee = ee[:, :512] + ee[:, 512:]
    eeo = ee[:, :512] - ee[:, 512:]
    eeee = eee[:, :256] + eee[:, 256:]
    eeeo = eee[:, :256] - eee[:, 256:]
    eeeee = eeee[:, :128] + eeee[:, 128:]
    eeeeo = eeee[:, :128] - eeee[:, 128:]
    parts = [eeeee, eeeeo]
    for sig in (eeeo, eeo, eo, d):
        parts.extend(_fold_cs(sig))
    q = np.concatenate(parts, axis=1).astype(ml_dtypes.float8_e4m3)
    # [128b+cr, 128s+p] -> [b, p, s, cr]
    arr = q.reshape(NB, 128, NSLOT, 128).transpose(0, 3, 2, 1)
    return {"crops": np.ascontiguousarray(arr)}


def _host_prepare(model_output, GT_sig, offsets_st, offsets_t):
    """Build per-core in_maps."""
    w_blob = _w_blob()
    from numpy.lib.stride_tricks import sliding_window_view
    in_maps = []
    mats = []   # 4 matrices' (e, d) row data [2048, 2048] each
    for b in range(B):
        offs = np.asarray(offsets_st[b], dtype=np.int64).reshape(-1)
        ch_idx = np.repeat(np.arange(C), K_CROPS)
        base = np.asarray(model_output[b], dtype=np.float32)
        win = sliding_window_view(base, L, axis=-1)  # [C, T-L+1, L]
        cr = win[ch_idx, offs]                       # [2048, L]
        mats.append((cr[:, :HALF] + cr[:, HALF:],
                     cr[:, :HALF] - cr[:, HALF:]))
    for b in range(B):
        offs = np.asarray(offsets_t[b], dtype=np.int64).reshape(-1)
        win = sliding_window_view(
            np.asarray(GT_sig[b], dtype=np.float32), L)
        cr = win[offs]
        mats.append((cr[:, :HALF] + cr[:, HALF:],
                     cr[:, :HALF] - cr[:, HALF:]))
    for m in range(4):
        e, d = mats[m]
        for h in range(2):
            sl = slice(h * ROWS_PER_CORE, (h + 1) * ROWS_PER_CORE)
            im = {"w": w_blob}
            im.update(_core_input((e[sl], d[sl])))
            in_maps.append(im)
    return in_maps


def _combine(results, label_flag):
    """results: list of 8 dicts with out_cs [1,F], out_rq [128,6*NB].

    out_rq holds bn_stats fields per block: [cnt_e, mean_e, cnt*var_e,
    cnt_o, mean_o, cnt*var_o] over the even/odd-indexed band bins of p.
    rowsum = cnt_e*mean_e + cnt_o*mean_o; sum(p^2) = (cnt*var + cnt*
    mean^2) summed over both halves.  Device cs used lhsT = 238/rowsum.
    """
    cs = np.zeros((4, F), dtype=np.float64)
    ssq = np.zeros(4, dtype=np.float64)
    for m in range(4):
        for h in range(2):
            r = results[2 * m + h]
            cs[m] += np.asarray(r["out_cs"], dtype=np.float64)[0] / 238.0
            rq = np.asarray(r["out_rq"], dtype=np.float64)
            ne, me, ve = rq[:, 0::6], rq[:, 1::6], rq[:, 2::6]
            no, mo, vo = rq[:, 3::6], rq[:, 4::6], rq[:, 5::6]
            rs = ne * me + no * mo
            q = (ve + ne * me * me) + (vo + no * mo * mo)
            ssq[m] += float(np.sum(q / (rs * rs)))

    N = float(N_ROWS)

    def cmp_excl(a):
        return (2.0 * N * ssq[a] - 2.0 * np.dot(cs[a], cs[a])) / F / (N * (N - 1.0))

    def cmp_full(a, b):
        return (N * ssq[a] + N * ssq[b] - 2.0 * np.dot(cs[a], cs[b])) / F / (N * N)

    lf = np.asarray(label_flag, dtype=np.float64).reshape(-1)
    lf_sum = lf[0] + lf[1]
    denom = 1.0 if lf_sum == 0 else lf_sum
    pos_loss = (cmp_excl(0) + cmp_excl(1)) / 2.0
    neg_loss = -cmp_full(0, 1)
    pos_GT = (lf[0] * cmp_full(0, 2) + lf[1] * cmp_full(1, 3)) / denom
    neg_GT = -(lf[0] * cmp_full(1, 2) + lf[1] * cmp_full(0, 3)) / denom
    if lf_sum == 0:
        pos_GT = 0.0
        neg_GT = 0.0
    loss = pos_loss + neg_loss + pos_GT + neg_GT
    return (np.float32(loss), np.float32(pos_loss), np.float32(neg_loss),
            np.float32(pos_GT), np.float32(neg_GT))


def run(inputs, trace=False):
    """Returns (outputs_tuple, BassKernelResults)."""
    from concourse import bass_utils
    nc = _build_module()
    in_maps = _host_prepare(
        inputs["model_output"], inputs["GT_sig"],
        inputs["offsets_st"], inputs["offsets_t"])
    res = bass_utils.run_bass_kernel_spmd(
        nc, in_maps, core_ids=list(range(N_CORES)), trace=trace)
    outs = _combine(res.results, inputs["label_flag"])
    return outs, res


def kernel(**inputs):
    outs, _ = run(inputs)
    return outs


# revision 21
# speedup vs baseline: 1.0769x; 1.0294x over previous
"""Trainium2 Bass kernel for nn_ContrastLoss (contrastive PSD loss).

Math notes (validated against the jax reference and a numpy emulator):
  * The band (rfft bins 92..568 of a 4096-point DFT) excludes DC, so the
    mean subtraction in the reference is a no-op for the band PSD.
  * diag(D) == 0 for the pairwise-MSE matrix, and every _compare() term
    reduces to rank-1 statistics of the normalized PSD matrices:
        sum_ij D_ij * F = M*SSQ_a + N*SSQ_b - 2 * cs_a . cs_b
    with SSQ = sum of squared entries and cs = column sums.  So the NxN
    Gram matrix is never materialized; the device only produces per-core
    column sums and per-row (sum, sum-of-squares) statistics.
  * Radix-2 DIF recursion on the crop halves splits the band bins into
    classes by k mod 32 with REAL sub-signals (x0+-x1 folds, applied
    where the class phases allow it):
        od    k odd       : d     [2048]   238 bins
        eo    k = 2 mod 4 : eo    [1024]   119 bins
        eeo   k = 4 mod 8 : eeo   [512]     60 bins
        eeeo  k = 8 mod 16: eeeo  [256]     30 bins
        eeeee k = 0 mod 32: eeeee [128]     15 bins
        eeeeo k =16 mod 32: eeeeo [128]     15 bins
  * Universal symmetric fold: for each class, cos(theta(n-m,k)) =
    -cos(theta(m,k)) and sin(theta(n-m,k)) = +sin(theta(m,k)), so
        Re X = [s_0, s_m - s_{n-m}] . cos-matrix     (n/2 contraction)
        Im X = [s_{n/2}, s_m + s_{n-m}] . -sin-matrix (n/2 contraction)
    (edge samples ride in the free m=0 slot; cos theta(n/2,k) = 0 and
    sin theta(0,k) = 0 for these classes).  This halves the matmul MACs
    again: 2620 PE cycles per 128-crop block, DFT matrices 655 KB.
  * All matmul operands are fp8-e4m3; >=256-contractions use DoubleRow
    perf mode (two 128-deep k-tiles per instruction).  End-to-end e4m3
    error: ~4e-6 on the loss terms, ~3e-3 on the cancellation-dominated
    total loss, far under the 2e-2 gate.

Device schedule per core (1024 crops of the 8192 total):
  Big-line DMAs (one descriptor per partition): W blob [128,2,2560]
  (5 KB lines) then 8 crop blocks [128,32,128] (4 KB lines).  Dummy fp8
  matmuls pre-ramp the PE p-state while DMA streams.  Per block: 20
  matmuls -> 2 PSUM tiles, ACT Square -> sq, DVE adds -> band PSD
  p [128,477], DVE row-reduce -> rowsum, ACT Square+accum -> rowsumsq,
  PE colsum matmul with lhsT = 1/rowsum -> cs [1,477].  Host combines
  the 8 cores' (cs, rowstats) in float64.
"""

import numpy as np

# Problem constants (hardcoded; kernel.py must be self-contained)
B, C, T = 2, 64, 32768
L = 4096
K_CROPS = 32
HALF = L // 2                  # 2048
N_ROWS = C * K_CROPS           # 2048 rows per PSD matrix
N_CORES = 8
ROWS_PER_CORE = N_ROWS * 4 // N_CORES   # 1024
NB = ROWS_PER_CORE // 128      # 8 row blocks per core
NSLOT = 32                     # 128-sample slots per crop
N_WARM = 22                    # PE p-state pre-ramp matmuls

# Band-bin classes: (kset, signal length n, folded?)
K_EEEEE = np.arange(96, 545, 32)    # 15 bins, k=0 mod 32
K_EEEEO = np.arange(112, 561, 32)   # 15 bins, k=16 mod 32
K_EEEO = np.arange(104, 569, 16)    # 30 bins, k=8 mod 16
K_EEO = np.arange(92, 565, 8)       # 60 bins, k=4 mod 8
K_EO = np.arange(94, 567, 4)        # 119 bins, k=2 mod 4
K_OD = np.arange(93, 568, 2)        # 238 bins, k odd
F = 477
FP = F + 1                     # pad col so fp32r matmul free dim is even
WX = 2560                      # W blob free columns (per k-tile slot)

_NC = None
_W_CACHE = None


def _w_blob():
    """fp8-e4m3 DFT matrices packed as [128, 2, WX].

    dim1 indexes the two k-tiles of a DoubleRow pair (or t=0/1 reuse for
    the four plain 128-contraction classes).  Layout in the X dim:
      [0:30)    t0 eeeee [cos|-sin], t1 eeeeo
      [30:60)   t0 eeeo-cos,         t1 eeeo-sin
      [60:120)  eeo-cos   [120:180) eeo-sin      (1 pair)
      [180:418) eo-cos x2 [418:656) eo-sin x2    (119 cols per pair)
      [656:1608) od-cos x4 [1608:2560) od-sin x4 (238 cols per pair)
    """
    global _W_CACHE
    if _W_CACHE is not None:
        return _W_CACHE
    import ml_dtypes

    def ang(ms, ks):
        return 2.0 * np.pi * (ms[:, None].astype(np.float64) *
                              ks[None, :]) / float(L)

    blob = np.zeros((128, 2, WX), dtype=np.float32)

    def put_plain(t, x0, ks):
        a = ang(np.arange(128), ks)
        blob[:, t, x0:x0 + 2 * len(ks)] = np.concatenate(
            [np.cos(a), -np.sin(a)], axis=1)

    def put_fold(x0c, x0s, ks, n):
        h = n // 2
        mc = np.concatenate([[0], np.arange(1, h)])
        ms = np.concatenate([[h], np.arange(1, h)])
        wc = np.cos(ang(mc, ks))           # [h, nb]
        ws = -np.sin(ang(ms, ks))
        npair = h // 256
        nb = len(ks)
        for w, x0 in ((wc, x0c), (ws, x0s)):
            r = w.reshape(npair, 2, 128, nb).transpose(2, 1, 0, 3)
            blob[:, :, x0:x0 + npair * nb] = r.reshape(128, 2, npair * nb)

    put_plain(0, 0, K_EEEEE)
    put_plain(1, 0, K_EEEEO)
    # eeeo: folded to 128-contraction -> plain slots t0/t1 at [30:60)
    h = 128
    mc = np.concatenate([[0], np.arange(1, h)])
    ms = np.concatenate([[h], np.arange(1, h)])
    blob[:, 0, 30:60] = np.cos(ang(mc, K_EEEO))
    blob[:, 1, 30:60] = -np.sin(ang(ms, K_EEEO))
    put_fold(60, 120, K_EEO, 512)
    put_fold(180, 418, K_EO, 1024)
    put_fold(656, 1608, K_OD, 2048)

    _W_CACHE = np.ascontiguousarray(blob.astype(ml_dtypes.float8_e4m3))
    return _W_CACHE


def _build_module():
    global _NC
    if _NC is not None:
        return _NC
    import concourse.bacc as bacc
    import concourse.bass as bass
    import concourse.tile as tile
    from concourse import mybir

    f32 = mybir.dt.float32
    f32r = mybir.dt.float32r
    f8 = mybir.dt.float8e4
    AF = mybir.ActivationFunctionType
    DR = mybir.MatmulPerfMode.DoubleRow

    nc = bacc.Bacc("TRN2", target_bir_lowering=False, debug=False,
                   num_devices=N_CORES)

    # crops: [blk, partition, slot, crop] fp8, 4 KB lines
    crops_d = nc.dram_tensor("crops", [NB, 128, NSLOT, 128], f8,
                             kind="ExternalInput")
    w_d = nc.dram_tensor("w", [128, 2, WX], f8, kind="ExternalInput")
    out_cs = nc.dram_tensor("out_cs", [1, F], f32, kind="ExternalOutput")
    out_rq = nc.dram_tensor("out_rq", [128, 6 * NB], f32,
                            kind="ExternalOutput")

    with tile.TileContext(nc) as tc:
        with (
            tc.tile_pool(name="wp", bufs=1) as wp,
            tc.tile_pool(name="cp", bufs=NB) as cp,
            tc.tile_pool(name="sq", bufs=4) as sqp,
            tc.tile_pool(name="pp", bufs=3) as ppool,
            tc.tile_pool(name="sm", bufs=8) as sm,
            tc.tile_pool(name="outp", bufs=1) as outp,
            tc.tile_pool(name="ps", bufs=6, space=bass.MemorySpace.PSUM) as ps,
            tc.tile_pool(name="psw", bufs=1, space=bass.MemorySpace.PSUM) as psw,
            tc.tile_pool(name="pcs", bufs=1, space=bass.MemorySpace.PSUM) as pcs,
        ):
            wb = wp.tile([128, 2, WX], f8)
            rq_t = outp.tile([128, 6 * NB], f32)
            zero_col = outp.tile([128, 1], f32)
            warm_t = outp.tile([128, 2, 256], f8)
            nc.vector.memset(zero_col, 0.0)
            nc.vector.memset(warm_t, 0.0)
            cs_psum = pcs.tile([1, FP], f32)
            warm_ps = psw.tile([128, 256], f32)

            # PE p-state pre-ramp while DMA streams.
            for i in range(N_WARM):
                nc.tensor.matmul(warm_ps, warm_t[:, :, 0:128], warm_t,
                                 start=True, stop=True, perf_mode=DR)

            # DMAs in consumption order (big descriptors, one hw queue)
            nc.sync.dma_start(out=wb, in_=w_d[:])
            cpb = []
            for blk in range(NB):
                ct = cp.tile([128, NSLOT, 128], f8, tag="cp", name=f"c{blk}")
                cpb.append(ct)
                nc.sync.dma_start(out=ct, in_=crops_d[blk])

            for blk in range(NB):
                cb = cpb[blk]
                ev_t = ps.tile([128, 478], f32, tag="ps", name=f"ev{blk}")
                od_t = ps.tile([128, 476], f32, tag="ps", name=f"od{blk}")
                # od first so the tail of the last block is the short
                # ev post-chain, not the long od one.
                for px0, s0, wx0 in ((0, 16, 656), (238, 24, 1608)):
                    for c in range(4):
                        nc.tensor.matmul(
                            od_t[:, px0:px0 + 238],
                            cb[:, s0 + 2 * c:s0 + 2 * c + 2, :],
                            wb[:, :, wx0 + 238 * c:wx0 + 238 * (c + 1)],
                            start=(c == 0), stop=(c == 3), perf_mode=DR)
                # ev_t layout: [eeeee c|s 0:30 | eeeeo c|s 30:60 |
                # cos-rest 60:269 | sin-rest 269:478]: the two 15-bin
                # classes keep merged 30-col matmuls (each matmul has a
                # ~100ns fixed floor, so fewer matmuls beats fewer DVE
                # adds), the rest pairs up via two big adds.
                for x0, slot, t, wx0 in ((0, 0, 0, 0), (30, 1, 1, 0),
                                         (60, 2, 0, 30), (269, 3, 1, 30)):
                    nc.tensor.matmul(ev_t[:, x0:x0 + 30], cb[:, slot, :],
                                     wb[:, t, wx0:wx0 + 30],
                                     start=True, stop=True)
                # DoubleRow folded ev classes
                for px0, s0, wx0, nb, npair in (
                        (90, 4, 60, 60, 1), (299, 6, 120, 60, 1),
                        (150, 8, 180, 119, 2), (359, 12, 418, 119, 2)):
                    for c in range(npair):
                        nc.tensor.matmul(
                            ev_t[:, px0:px0 + nb],
                            cb[:, s0 + 2 * c:s0 + 2 * c + 2, :],
                            wb[:, :, wx0 + nb * c:wx0 + nb * (c + 1)],
                            start=(c == 0), stop=(c == npair - 1),
                            perf_mode=DR)

                # Post: PSD, row stats (bn_stats), normalized column sums.
                sq_ev = sqp.tile([128, 478], f32, tag="sqe", name=f"se{blk}")
                sq_od = sqp.tile([128, 476], f32, tag="sqd", name=f"sd{blk}")
                nc.scalar.activation(out=sq_od, in_=od_t, func=AF.Square)
                nc.scalar.activation(out=sq_ev, in_=ev_t, func=AF.Square)
                p_t = ppool.tile([128, FP], f32r, tag="p", name=f"p{blk}")
                with nc.allow_low_precision(reason="fp32r is fp32-width"):
                    nc.vector.tensor_add(p_t[:, 239:477], sq_od[:, 0:238],
                                         sq_od[:, 238:476])
                    nc.vector.tensor_add(p_t[:, 0:15], sq_ev[:, 0:15],
                                         sq_ev[:, 15:30])
                    nc.vector.tensor_add(p_t[:, 15:30], sq_ev[:, 30:45],
                                         sq_ev[:, 45:60])
                    nc.vector.tensor_add(p_t[:, 30:239], sq_ev[:, 60:269],
                                         sq_ev[:, 269:478])
                    nc.vector.tensor_copy(p_t[:, F:FP], zero_col)
                # bn_stats: [cnt_e, mean_e, cnt*var_e, cnt_o, mean_o,
                # cnt*var_o] over even/odd-indexed band bins; the host
                # reconstructs rowsum and sum(p^2) exactly.
                bn = rq_t[:, 6 * blk:6 * blk + 6]
                nc.vector.bn_stats(bn, p_t[:, 0:F])
                # rs/238 = mean_e*(239/238) + mean_o; colsum lhsT is then
                # 238/rs and the host divides cs by 238.
                rsn = sm.tile([128, 1], f32, tag="rsn", name=f"r{blk}")
                nc.vector.scalar_tensor_tensor(
                    out=rsn, in0=bn[:, 1:2], scalar=239.0 / 238.0,
                    in1=bn[:, 4:5], op0=mybir.AluOpType.mult,
                    op1=mybir.AluOpType.add)
                inv = sm.tile([128, 1], f32r, tag="inv", name=f"i{blk}")
                with nc.allow_low_precision(reason="fp32r is fp32-width"):
                    nc.vector.reciprocal(inv, rsn)
                nc.tensor.matmul(cs_psum, inv, p_t,
                                 start=(blk == 0), stop=(blk == NB - 1))

            cs_sb = outp.tile([1, F], f32)
            nc.vector.tensor_copy(cs_sb, cs_psum[:, 0:F])
            nc.sync.dma_start(out=out_cs[:], in_=cs_sb)
            nc.sync.dma_start(out=out_rq[:], in_=rq_t)

    nc.compile()
    _NC = nc
    return nc


def _fold_cs(sig):
    """sig [R, n] -> (cos fold [R, n/2], sin fold [R, n/2])."""
    n = sig.shape[1]
    h = n // 2
    c = np.empty((sig.shape[0], h), dtype=np.float32)
    s = np.empty_like(c)
    c[:, 0] = sig[:, 0]
    s[:, 0] = sig[:, h]
    c[:, 1:] = sig[:, 1:h] - sig[:, :h:-1]
    s[:, 1:] = sig[:, 1:h] + sig[:, :h:-1]
    return c, s


def _core_input(rows_ed):
    """rows_ed: (e, d) each [1024, 2048] f32 -> crops [8,128,32,128] fp8."""
    import ml_dtypes
    e, d = rows_ed
    ee = e[:, :1024] + e[:, 1024:]
    eo = e[:, :1024] - e[:, 1024:]
    eee = ee[:, :512] + ee[:, 512:]
    eeo = ee[:, :512] - ee[:, 512:]
    eeee = eee[:, :256] + eee[:, 256:]
    eeeo = eee[:, :256] - eee[:, 256:]
    eeeee = eeee[:, :128] + eeee[:, 128:]
    eeeeo = eeee[:, :128] - eeee[:, 128:]
    parts = [eeeee, eeeeo]
    for sig in (eeeo, eeo, eo, d):
        parts.extend(_fold_cs(sig))
    q = np.concatenate(parts, axis=1).astype(ml_dtypes.float8_e4m3)
    # [128b+cr, 128s+p] -> [b, p, s, cr]
    arr = q.reshape(NB, 128, NSLOT, 128).transpose(0, 3, 2, 1)
    return {"crops": np.ascontiguousarray(arr)}


def _host_prepare(model_output, GT_sig, offsets_st, offsets_t):
    """Build per-core in_maps."""
    w_blob = _w_blob()
    from numpy.lib.stride_tricks import sliding_window_view
    in_maps = []
    mats = []   # 4 matrices' (e, d) row data [2048, 2048] each
    for b in range(B):
        offs = np.asarray(offsets_st[b], dtype=np.int64).reshape(-1)
        ch_idx = np.repeat(np.arange(C), K_CROPS)
        base = np.asarray(model_output[b], dtype=np.float32)
        win = sliding_window_view(base, L, axis=-1)  # [C, T-L+1, L]
        cr = win[ch_idx, offs]                       # [2048, L]
        mats.append((cr[:, :HALF] + cr[:, HALF:],
                     cr[:, :HALF] - cr[:, HALF:]))
    for b in range(B):
        offs = np.asarray(offsets_t[b], dtype=np.int64).reshape(-1)
        win = sliding_window_view(
            np.asarray(GT_sig[b], dtype=np.float32), L)
        cr = win[offs]
        mats.append((cr[:, :HALF] + cr[:, HALF:],
                     cr[:, :HALF] - cr[:, HALF:]))
    for m in range(4):
        e, d = mats[m]
        for h in range(2):
            sl = slice(h * ROWS_PER_CORE, (h + 1) * ROWS_PER_CORE)
            im = {"w": w_blob}
            im.update(_core_input((e[sl], d[sl])))
            in_maps.append(im)
    return in_maps


def _combine(results, label_flag):
    """results: list of 8 dicts with out_cs [1,F], out_rq [128,6*NB].

    out_rq holds bn_stats fields per block: [cnt_e, mean_e, cnt*var_e,
    cnt_o, mean_o, cnt*var_o] over the even/odd-indexed band bins of p.
    rowsum = cnt_e*mean_e + cnt_o*mean_o; sum(p^2) = (cnt*var + cnt*
    mean^2) summed over both halves.  Device cs used lhsT = 238/rowsum.
    """
    cs = np.zeros((4, F), dtype=np.float64)
    ssq = np.zeros(4, dtype=np.float64)
    for m in range(4):
        for h in range(2):
            r = results[2 * m + h]
            cs[m] += np.asarray(r["out_cs"], dtype=np.float64)[0] / 238.0
            rq = np.asarray(r["out_rq"], dtype=np.float64)
            ne, me, ve = rq[:, 0::6], rq[:, 1::6], rq[:, 2::6]
            no, mo, vo = rq[:, 3::6], rq[:, 4::6], rq[:, 5::6]
            rs = ne * me + no * mo
            q = (ve + ne * me * me) + (vo + no * mo * mo)
            ssq[m] += float(np.sum(q / (rs * rs)))

    N = float(N_ROWS)

    def cmp_excl(a):
        return (2.0 * N * ssq[a] - 2.0 * np.dot(cs[a], cs[a])) / F / (N * (N - 1.0))

    def cmp_full(a, b):
        return (N * ssq[a] + N * ssq[b] - 2.0 * np.dot(cs[a], cs[b])) / F / (N * N)

    lf = np.asarray(label_flag, dtype=np.float64).reshape(-1)
    lf_sum = lf[0] + lf[1]
    denom = 1.0 if lf_sum == 0 else lf_sum
    pos_loss = (cmp_excl(0) + cmp_excl(1)) / 2.0
    neg_loss = -cmp_full(0, 1)
    pos_GT = (lf[0] * cmp_full(0, 2) + lf[1] * cmp_full(1, 3)) / denom
    neg_GT = -(lf[0] * cmp_full(1, 2) + lf[1] * cmp_full(0, 3)) / denom
    if lf_sum == 0:
        pos_GT = 0.0
        neg_GT = 0.0
    loss = pos_loss + neg_loss + pos_GT + neg_GT
    return (np.float32(loss), np.float32(pos_loss), np.float32(neg_loss),
            np.float32(pos_GT), np.float32(neg_GT))


def run(inputs, trace=False):
    """Returns (outputs_tuple, BassKernelResults)."""
    from concourse import bass_utils
    nc = _build_module()
    in_maps = _host_prepare(
        inputs["model_output"], inputs["GT_sig"],
        inputs["offsets_st"], inputs["offsets_t"])
    res = bass_utils.run_bass_kernel_spmd(
        nc, in_maps, core_ids=list(range(N_CORES)), trace=trace)
    outs = _combine(res.results, inputs["label_flag"])
    return outs, res


def kernel(**inputs):
    outs, _ = run(inputs)
    return outs
